# revision 60
# baseline (speedup 1.0000x reference)
"""Trainium2 Bass kernel for GRU model (nn_Model_1331439862409).

Model: tokens [B=512, S=512] -> embedding [30522, 100] -> single-layer GRU
(hidden 512) scanned over S -> final hidden state -> linear [512 -> 2].

Sharding: data-parallel over 8 NeuronCores (64 batch rows per core);
embedding table + weights replicated; the sequential scan stays local.

Per-core design. The wall time is set by the per-step serial dependency
loop (PE gh -> sigmoid -> t -> u -> tanh -> a2 -> PE), so the kernel
minimizes that loop and runs two phase-shifted batch streams of NB=32
rows so each stream's stalls hide under the other's engine work:
  - gh for r/z splits into two rhs streams (h = a2p - c1neg):
    c1neg = -z*h_prev (fp8 DoubleRow, off the critical loop) and
    a2p = (1-z)*n (fp8 DoubleRow - the only post-tanh matmul leg).
  - The z-gate weight rows are NEGATED so the sigmoid directly yields
    w = 1-z (sigma(-x)): a2p = w*n and c1neg = (w-1)*h_prev become plain
    two-operand ops with no extra "1-z" instruction.
  - gx for r/z: fp8 DoubleRow with a residual-compensation stream; the fp8
    table is padded to 256 cols so the 16-bit-granularity transpose gather
    lands byte pairs (dims 2p/2p+1) on partition p, matching DoubleRow's
    [K,2,N] rhs layout directly.
  - All rz-path weights are scaled x64; the sigmoid descales by 1/64.
  - n gate: full fp16 (its value feeds h directly): gh_n from h16.
  - u = xn + r*hn is computed ON THE PE: an identity matmul accumulates
    t = r*hn into the xn PSUM bank (hn and xn live in separate banks so
    t can read hn after its accumulation group closes).
  - Biases ride matmuls: table col 100 = 1.0; b_hh_n broadcasts via a tiny
    [4,x] matmul into the hn PSUM bank.
  - Engine split per stream-step: ACT sigmoid(rz)+tanh; DVE t, a2p(f8),
    a2p(f16), h', wm1=w-1 (TensorScalarPtr is illegal on Pool, so the
    -z factor is materialized on DVE); PE u-accumulate; Pool c1neg(f8),
    c1neg(f16) + the embedding gathers.
"""

import numpy as np
import ml_dtypes
from contextlib import ExitStack

import concourse.bass as bass
import concourse.mybir as mybir
import concourse.tile as tile
from concourse import bacc
from concourse.bass_utils import run_bass_kernel_spmd

F16 = mybir.dt.float16
F32 = mybir.dt.float32
F8 = mybir.dt.float8e4
I16 = mybir.dt.int16
AF = mybir.ActivationFunctionType
OP = mybir.AluOpType
DR = mybir.MatmulPerfMode.DoubleRow
NP8 = ml_dtypes.float8_e4m3fn

VOCAB, EMB, HID, OUT = 30522, 100, 512, 2
B, S = 512, 512
NCORES = 8
BL = B // NCORES          # 64 batch rows per core
NB = 32                   # rows per stream (2 streams)
NK = 4                    # hidden chunks of 128
NKP = 2                   # DoubleRow ktile pairs (256 each)
NMRZ = 8                  # r/z gate-row chunks of 128
NMN = 4                   # n gate-row chunks of 128
WSCALE = 64.0             # rz-path weight scale
GCH_STEPS = 64            # timesteps per gather chunk
GCH = GCH_STEPS * BL      # tokens per gather chunk (4096)
PART2_RESID = False       # fp8 residual stream on the a2m (chain) leg


def build_program(s_steps=S):
    n_tok = s_steps * BL
    n_chunks = (n_tok + GCH - 1) // GCH

    nc = bacc.Bacc("TRN2", target_bir_lowering=False, debug=False)

    table16 = nc.dram_tensor("table16", [VOCAB, 128], F16, kind="ExternalInput")
    table8 = nc.dram_tensor("table8", [VOCAB, 256], F8, kind="ExternalInput")
    idx = nc.dram_tensor("idx", [128, n_tok // 16], I16, kind="ExternalInput")
    wxrz = nc.dram_tensor("wxrz", [128, 2, NMRZ, 2, 128], F8, kind="ExternalInput")
    whrzc = nc.dram_tensor("whrzc", [128, 2, NMRZ, NKP, 2, 128], F8,
                           kind="ExternalInput")
    nv = 2 if PART2_RESID else 1
    whrza = nc.dram_tensor("whrza", [128, nv, NMRZ, NKP, 2, 128], F8,
                           kind="ExternalInput")
    wxn = nc.dram_tensor("wxn", [128, NMN, 128], F16, kind="ExternalInput")
    whn = nc.dram_tensor("whn", [128, NMN, NK, 128], F16, kind="ExternalInput")
    bhn = nc.dram_tensor("bhn", [NK, 128], F16, kind="ExternalInput")
    blkones = nc.dram_tensor("blkones", [NK, NK * NB], F16, kind="ExternalInput")
    fcw = nc.dram_tensor("fcw", [128, NK, OUT], F32, kind="ExternalInput")
    fcb = nc.dram_tensor("fcb", [1, OUT], F32, kind="ExternalInput")
    eye = nc.dram_tensor("eye", [128, 128], F16, kind="ExternalInput")
    out = nc.dram_tensor("out", [BL, OUT], F32, kind="ExternalOutput")

    with tile.TileContext(nc) as tc, ExitStack() as ctx:
        const = ctx.enter_context(tc.tile_pool(name="const", bufs=1))
        embp = ctx.enter_context(tc.tile_pool(name="emb", bufs=1))
        hp = ctx.enter_context(tc.tile_pool(name="h", bufs=2))
        gates = ctx.enter_context(tc.tile_pool(name="gates", bufs=2))
        przp = [ctx.enter_context(tc.tile_pool(name=f"prz{s}", bufs=1, space="PSUM"))
                for s in range(2)]
        hnp = [ctx.enter_context(tc.tile_pool(name=f"hn{s}", bufs=1, space="PSUM"))
               for s in range(2)]
        xnp = [ctx.enter_context(tc.tile_pool(name=f"xn{s}", bufs=1, space="PSUM"))
               for s in range(2)]
        pout = ctx.enter_context(tc.tile_pool(name="pout", bufs=1, space="PSUM"))

        # ---- constants into SBUF ----
        wxrz_sb = const.tile([128, 2, NMRZ, 2, 128], F8)
        nc.sync.dma_start(wxrz_sb[:], wxrz.ap())
        whrzc_sb = const.tile([128, 2, NMRZ, NKP, 2, 128], F8)
        nc.sync.dma_start(whrzc_sb[:], whrzc.ap())
        whrza_sb = const.tile([128, nv, NMRZ, NKP, 2, 128], F8)
        nc.sync.dma_start(whrza_sb[:], whrza.ap())
        wxn_sb = const.tile([128, NMN, 128], F16)
        nc.sync.dma_start(wxn_sb[:], wxn.ap())
        whn_sb = const.tile([128, NMN, NK, 128], F16)
        nc.sync.dma_start(whn_sb[:], whn.ap())
        bhn_sb = const.tile([NK, 128], F16)
        nc.sync.dma_start(bhn_sb[:], bhn.ap())
        blk_sb = const.tile([NK, NK * NB], F16)
        nc.sync.dma_start(blk_sb[:], blkones.ap())
        fcw_sb = const.tile([128, NK, OUT], F32)
        nc.sync.dma_start(fcw_sb[:], fcw.ap())
        fcb_sb = const.tile([1, OUT], F32)
        nc.sync.dma_start(fcb_sb[:], fcb.ap())
        eye_sb = const.tile([128, 128], F16)
        nc.sync.dma_start(eye_sb[:], eye.ap())
        ones1 = const.tile([1, BL], F32)
        nc.vector.memset(ones1[:], 1.0)
        idx_sb = const.tile([128, n_tok // 16], I16)
        nc.sync.dma_start(idx_sb[:], idx.ap())
        h32 = const.tile([128, NK, 2, NB], F32)

        # ---- embedding gathers (SWDGE); first chunks upfront, rest staggered
        emb16_t = [None] * n_chunks
        emb8_t = [None] * n_chunks

        def emit_gathers(c):
            nw = min(GCH, n_tok - c * GCH)
            e16 = embp.tile([128, 1, GCH], F16, tag=f"e16_{c}")
            nc.gpsimd.dma_gather(
                out_ap=e16[:, :, :nw], in_ap=table16.ap(),
                idxs_ap=idx_sb[:, c * (GCH // 16):c * (GCH // 16) + nw // 16],
                num_idxs=nw, num_idxs_reg=nw, elem_size=128, transpose=True,
                single_packet=(nw * 256 // 8 <= 16384))
            e8 = embp.tile([128, 2, nw], F8, tag=f"e8_{c}")
            nc.gpsimd.dma_gather(
                out_ap=e8[:], in_ap=table8.ap(),
                idxs_ap=idx_sb[:, c * (GCH // 16):c * (GCH // 16) + nw // 16],
                num_idxs=nw, num_idxs_reg=nw, elem_size=256, transpose=True,
                single_packet=(nw * 256 // 8 <= 16384))
            emb16_t[c] = e16
            emb8_t[c] = e8

        emit_gathers(0)
        if n_chunks > 1:
            emit_gathers(1)

        # ---- per-stream state ----
        hprev16 = [None, None]   # h16(t-1)
        c1prev = [None, None]    # c1_16(t-1) = z(t-1) * h16(t-2)
        c1prev8 = [None, None]   # fp8 copy for the part1 DoubleRow
        a2prev8 = [None, None]   # a2m_8(t-1) = (z-1)*n in fp8
        wm1_t = [None, None]
        r16_t = [None, None]
        z16_t = [None, None]
        u16_t = [None, None]
        n16_t = [None, None]
        prz_t = [None, None]
        hn_t = [None, None]
        xn_t = [None, None]
        t16_t = [None, None]

        def et16(s, t):
            c, off = divmod(t, GCH_STEPS)
            c0 = off * BL + NB * s
            return emb16_t[c][:, 0, c0:c0 + NB]

        def et8(s, t):
            c, off = divmod(t, GCH_STEPS)
            i0 = off * BL + NB * s
            flat = emb8_t[c][:].rearrange("p a b -> p (a b)")
            return flat[:, 2 * i0:2 * i0 + 2 * NB].rearrange(
                "p (i j) -> p j i", j=2)

        def f_rz(s, t):
            """PE gx (fp8 DR) + gh part1 (fp16 via c1) + part2 (fp8 via a2m);
            then sigmoid r (chain) and sigmoid z."""
            prz = przp[s].tile([128, 2 * NK * NB], F32, tag=f"prz{s}")
            prz_t[s] = prz
            e8 = et8(s, t)
            last_is_gx = t == 0
            for m in range(NMRZ):
                sl = prz[:, NB * m:NB * m + NB]
                nc.tensor.matmul(sl, lhsT=wxrz_sb[:, 0, m], rhs=e8,
                                 start=(m == 0), stop=False, perf_mode=DR)
                nc.tensor.matmul(sl, lhsT=wxrz_sb[:, 1, m], rhs=e8,
                                 start=False,
                                 stop=(last_is_gx and m == NMRZ - 1),
                                 perf_mode=DR)
            if t >= 2:
                c18 = c1prev8[s]
                for m in range(NMRZ):
                    sl = prz[:, NB * m:NB * m + NB]
                    for pp in range(NKP):
                        rhs = c18[:, 2 * pp:2 * pp + 2, :]
                        for v in range(2):
                            nc.tensor.matmul(sl, lhsT=whrzc_sb[:, v, m, pp],
                                             rhs=rhs, start=False, stop=False,
                                             perf_mode=DR)
            if t >= 1:
                a2 = a2prev8[s]
                for m in range(NMRZ):
                    sl = prz[:, NB * m:NB * m + NB]
                    for pp in range(NKP):
                        rhs = a2[:, 2 * pp:2 * pp + 2, :]
                        for v in range(nv):
                            nc.tensor.matmul(
                                sl, lhsT=whrza_sb[:, v, m, pp], rhs=rhs,
                                start=False,
                                stop=(m == NMRZ - 1 and pp == NKP - 1
                                      and v == nv - 1),
                                perf_mode=DR)
        def f_sigr(s):
            rz16 = gates.tile([128, 2 * NK * NB], F16, tag=f"rz{s}")
            nc.scalar.activation(rz16[:], prz_t[s][:], AF.Sigmoid,
                                 scale=1.0 / WSCALE)
            r16_t[s] = rz16[:, 0:NK * NB]
            z16_t[s] = rz16[:, NK * NB:2 * NK * NB]

        def f_sigz(s):
            pass

        def f_phx(s, t):
            """PE bias+gh_n (hn bank) + gx_n (xn bank; group stays open for
            the identity-matmul u accumulation)."""
            first = t == 0
            hn = hnp[s].tile([128, NK * NB], F32, tag=f"hn{s}")
            xn = xnp[s].tile([128, NK * NB], F32, tag=f"xn{s}")
            hn_t[s], xn_t[s] = hn, xn
            nc.tensor.matmul(hn[:], lhsT=bhn_sb[:], rhs=blk_sb[:],
                             start=True, stop=first)
            e16 = et16(s, t)
            for mi in range(NMN):
                nc.tensor.matmul(xn[:, NB * mi:NB * mi + NB],
                                 lhsT=wxn_sb[:, mi], rhs=e16,
                                 start=(mi == 0), stop=False)
            if not first:
                h16 = hprev16[s]
                for mi in range(NMN):
                    sl = hn[:, NB * mi:NB * mi + NB]
                    for k in range(NK):
                        nc.tensor.matmul(sl, lhsT=whn_sb[:, mi, k],
                                         rhs=h16[:, k, :], start=False,
                                         stop=(mi == NMN - 1 and k == NK - 1))

        def f_tu(s):
            """DVE t (the chain mid-leg); u happens on PE via identity."""
            t16 = gates.tile([128, NK * NB], F16, tag=f"t{s}")
            nc.vector.tensor_mul(t16[:], r16_t[s], hn_t[s][:])
            t16_t[s] = t16

        def f_umm(s):
            """PE: xn += I @ t (closes the xn accumulation group)."""
            nc.tensor.matmul(xn_t[s][:], lhsT=eye_sb[:], rhs=t16_t[s][:],
                             start=False, stop=True)

        def f_tanh(s):
            n16 = gates.tile([128, NK * NB], F16, tag=f"n{s}")
            nc.scalar.activation(n16[:], xn_t[s][:], AF.Tanh)
            n16_t[s] = n16

        def f_wm1(s):
            """DVE: wm1 = w - 1 = -z (off-chain; Pool TTs need it since
            TensorScalarPtr is not legal on the Pool engine)."""
            wm1 = gates.tile([128, NK * NB], F16, tag=f"wm1_{s}")
            nc.vector.tensor_scalar_add(wm1[:], z16_t[s], -1.0)
            wm1_t[s] = wm1

        def f_c1(s, t):
            """Pool: c1neg = wm1*h_prev = -z*h_prev, in fp8 (part1 rhs)
            and f16 (feeds h')."""
            if t == 0:
                return
            wm1 = wm1_t[s][:]
            c18 = hp.tile([128, NK, NB], F8, tag=f"c18_{s}")
            nc.gpsimd.tensor_mul(
                c18[:].rearrange("p a b -> p (a b)"), wm1,
                hprev16[s][:].rearrange("p a b -> p (a b)"))
            c1prev8[s] = c18
            c1 = hp.tile([128, NK, NB], F16, tag=f"c1_{s}")
            nc.gpsimd.tensor_mul(
                c1[:].rearrange("p a b -> p (a b)"), wm1,
                hprev16[s][:].rearrange("p a b -> p (a b)"))
            c1prev[s] = c1

        def f_tail_dve(s, t, last):
            """DVE per-stream tail: a2p_8 (chain), a2p_16, h'.
            a2p = w*n with w = 1-z; h' = a2p - c1neg."""
            first = t == 0
            n = n16_t[s][:]
            w = z16_t[s]
            a28 = hp.tile([128, NK, NB], F8, tag=f"a28_{s}")
            nc.vector.tensor_mul(a28[:].rearrange("p a b -> p (a b)"), w, n)
            a2prev8[s] = a28
            a216 = gates.tile([128, NK, NB], F16, tag=f"a216_{s}")
            nc.vector.tensor_mul(
                a216[:].rearrange("p a b -> p (a b)"), w, n)
            if last:
                hnew = h32[:, :, s, :]
            else:
                h16 = hp.tile([128, NK, NB], F16, tag=f"h16_{s}")
                hnew = h16[:]
            if first:
                nc.vector.tensor_copy(hnew, a216[:])
            else:
                nc.vector.tensor_sub(hnew, a216[:], c1prev[s][:])
            if not last:
                hprev16[s] = h16

        # ---- the scan ----
        for t in range(s_steps):
            last = t == s_steps - 1
            cnext = t // GCH_STEPS + 2
            if t % GCH_STEPS == 8 and cnext < n_chunks:
                emit_gathers(cnext)
            for s in range(2):
                f_phx(s, t)
            for s in range(2):
                f_rz(s, t)
            for s in range(2):
                f_sigr(s)
            for s in range(2):
                f_sigz(s)
            for s in range(2):
                f_tu(s)
            for s in range(2):
                f_umm(s)
            for s in range(2):
                f_wm1(s)
            for s in range(2):
                f_c1(s, t)
            for s in range(2):
                f_tanh(s)
            for s in range(2):
                f_tail_dve(s, t, last)

        # ---- final projection: out = h @ fc_w.T + fc_b ----
        pout_t = pout.tile([BL, OUT], F32)
        for k in range(NK):
            nc.tensor.matmul(
                pout_t[:], lhsT=h32[:, k].rearrange("p a b -> p (a b)"),
                rhs=fcw_sb[:, k], start=(k == 0), stop=False)
        nc.tensor.matmul(pout_t[:], lhsT=ones1[:], rhs=fcb_sb[:],
                         start=False, stop=True)
        out_sb = const.tile([BL, OUT], F32)
        nc.vector.tensor_copy(out_sb[:], pout_t[:])
        nc.sync.dma_start(out.ap(), out_sb[:])

    nc.finalize()
    return nc


def _q8_pair(w):
    """fp8 main+resid quantization at WSCALE: w -> (main, resid)."""
    ws = (w * WSCALE).astype(np.float32)
    main = ws.astype(NP8)
    resid = (ws - main.astype(np.float32)).astype(NP8)
    return main, resid


def prep_shared(embed_table, w_ih, w_hh, b_ih, b_hh, fc_w, fc_b):
    """Host-side weight prepacking (replicated across cores)."""
    embed_table = np.asarray(embed_table, dtype=np.float32)
    w_ih = np.asarray(w_ih, dtype=np.float32)
    w_hh = np.asarray(w_hh, dtype=np.float32)
    b_ih = np.asarray(b_ih, dtype=np.float32)
    b_hh = np.asarray(b_hh, dtype=np.float32)

    table16 = np.zeros((VOCAB, 128), dtype=np.float16)
    table16[:, :EMB] = embed_table.astype(np.float16)
    table16[:, EMB] = 1.0

    t8 = np.zeros((VOCAB, 256), dtype=np.float32)
    t8[:, :EMB] = embed_table
    t8[:, EMB] = 1.0
    table8 = t8.astype(NP8)

    # rz input weights (+bias row 100 = b_ih+b_hh), fp8 x64 main+resid
    waug = np.zeros((2 * HID, 256), dtype=np.float32)
    waug[:, :EMB] = w_ih[:2 * HID]
    waug[:, EMB] = b_ih[:2 * HID] + b_hh[:2 * HID]
    waug[HID:] *= -1.0          # z rows negated: sigma gives w = 1-z
    main, resid = _q8_pair(waug)
    wxrz = np.zeros((128, 2, NMRZ, 2, 128), dtype=NP8)
    for v, arr in ((0, main), (1, resid)):
        a = arr.reshape(NMRZ, 128, 128, 2)      # [m, c, p, j]
        wxrz[:, v] = a.transpose(2, 0, 3, 1)     # [p, m, j, c]

    # part1 (c1neg stream): rhs is -z*h, so r rows need -W, z rows +W
    # (z-row preacts are negated): whrzc = [-64*W_r; +64*W_z]
    whc = w_hh[:2 * HID].copy()
    whc[:HID] *= -1.0
    main, resid = _q8_pair(whc)
    whrzc = np.zeros((128, 2, NMRZ, NKP, 2, 128), dtype=NP8)
    for v, arr in ((0, main), (1, resid)):
        a = arr.reshape(NMRZ, 128, NKP, 2, 128)
        whrzc[:, v] = a.transpose(4, 0, 2, 3, 1)

    # part2 (a2m stream): fp8 NEGATED x64 weights (+optional resid):
    # whrza[p, v, m, pp, i, c] = -64*W_hh[128m+c, 256pp+128i+p]
    nv = 2 if PART2_RESID else 1
    # part2 (a2p stream): rhs +a2p: [+64*W_r; -64*W_z]
    wha = w_hh[:2 * HID].copy()
    wha[HID:] *= -1.0
    main, resid = _q8_pair(wha)
    whrza = np.zeros((128, nv, NMRZ, NKP, 2, 128), dtype=NP8)
    for v, arr in ((0, main), (1, resid))[:nv]:
        a = arr.reshape(NMRZ, 128, NKP, 2, 128)  # [m, c, pp, i, p]
        whrza[:, v] = a.transpose(4, 0, 2, 3, 1)  # [p, m, pp, i, c]

    wnaug = np.zeros((HID, 128), dtype=np.float32)
    wnaug[:, :EMB] = w_ih[2 * HID:]
    wnaug[:, EMB] = b_ih[2 * HID:]
    wxn = wnaug.reshape(NMN, 128, 128).transpose(2, 0, 1).astype(np.float16).copy()

    whn = (w_hh[2 * HID:].reshape(NMN, 128, NK, 128)
           .transpose(3, 0, 2, 1).astype(np.float16).copy())

    bhn = b_hh[2 * HID:].astype(np.float16).reshape(NK, 128).copy()
    blk = np.zeros((NK, NK * NB), dtype=np.float16)
    for c in range(NK):
        blk[c, NB * c:NB * c + NB] = 1.0
    fcw = np.asarray(fc_w, np.float32).T.reshape(NK, 128, OUT).transpose(1, 0, 2).copy()
    fcb = np.asarray(fc_b, np.float32).reshape(1, OUT)
    return dict(table16=table16, table8=table8, wxrz=wxrz, whrzc=whrzc,
                whrza=whrza, wxn=wxn, whn=whn, bhn=bhn, blkones=blk,
                fcw=fcw, fcb=fcb, eye=np.eye(128, dtype=np.float16))


def prep_idx(x_core, s_steps):
    """Token indices in (t, b) order: [128, n_tok//16] int16."""
    n_tok = s_steps * BL
    toks = x_core[:, :s_steps].T.ravel().astype(np.int64)
    assert toks.max() < VOCAB
    idx_np = np.zeros((128, n_tok // 16), dtype=np.int16)
    for c in range((n_tok + GCH - 1) // GCH):
        nw = min(GCH, n_tok - c * GCH)
        chunk = toks[c * GCH:c * GCH + nw].reshape(nw // 16, 16).T
        idx_np[:, c * (GCH // 16):c * (GCH // 16) + nw // 16] = np.tile(
            chunk.astype(np.int16), (8, 1))
    return idx_np


_PROG_CACHE = {}


def kernel(x, embed_table, w_ih, w_hh, b_ih, b_hh, fc_w, fc_b, _s_steps=S,
           _trace=False):
    x = np.asarray(x)
    s_steps = _s_steps

    if s_steps not in _PROG_CACHE:
        _PROG_CACHE[s_steps] = build_program(s_steps)
    nc = _PROG_CACHE[s_steps]

    shared = prep_shared(embed_table, w_ih, w_hh, b_ih, b_hh, fc_w, fc_b)

    in_maps = []
    for core in range(NCORES):
        xc = x[BL * core:BL * (core + 1), :]
        m = dict(shared)
        m["idx"] = prep_idx(xc, s_steps)
        in_maps.append(m)

    res = run_bass_kernel_spmd(nc, in_maps, core_ids=list(range(NCORES)),
                               trace=_trace)
    out = np.concatenate([res.results[i]["out"] for i in range(NCORES)], axis=0)
    if _trace:
        kernel.last_exec_time_ns = res.exec_time_ns
        kernel.last_results = res
    return out.astype(np.float32)
